# revision 28
# baseline (speedup 1.0000x reference)
"""MoE FFN (8 experts, top-2) Trainium2 Bass kernel.

Strategy: EXPERT-parallel across the 8 cores. Core e owns expert e's
weights, resident in SBUF as bf16 (w1 64KiB/partition + w2 64KiB/partition),
and processes every token routed to expert e (CAP=2176 padded slots; actual
max expert load for this input distribution is ~2120 of mean 2048). The tiny
router (0.06% of FLOPs) runs on host in exact fp32 (matching the reference's
op order so top-2 selection is stable); the host also gathers/transposes each
expert's tokens and scatter-gathers the outputs back, so the device kernel is
a pure dense FFN: hT = gelu_tanh(w1 @ x + b1) -> y = gate * (hT.T @ w2 + b2).

This beats token-parallel (capacity 384 per (core,expert) = 3072 slots/core,
1.5x padding) at 2176 slots/core (1.06x padding), removes all device-side
gathers/transposes/scatters, and streams weights exactly once (resident).

Everything stays in [feature, token] layout on chip: fc1 consumes w1T tiles
as stationary and xT as moving; fc2 consumes hT tiles as stationary and w2T
as moving, producing [token, d] PSUM tiles so the per-token gate is a
per-partition scalar multiply (one DVE op). b2 is added via a K=1 ones
matmul folded into the fc2 PSUM accumulation. Matmul operands are bf16
(fp32 PSUM accumulate): rel err ~5e-3, well under the 2e-2 gate.
"""

import numpy as np
from contextlib import ExitStack

import ml_dtypes

import concourse.bass as bass
import concourse.bacc as bacc
import concourse.tile as tile
from concourse import mybir
from concourse.bass_utils import run_bass_kernel_spmd

BF16 = ml_dtypes.bfloat16
FR = mybir.dt.float32r
BF = mybir.dt.bfloat16
F32 = mybir.dt.float32
AF = mybir.ActivationFunctionType
OP = mybir.AluOpType

NCORES = 8
E = 8            # experts
D = 1024         # model dim
H = 4096         # hidden dim
DS = D // 128    # d sub-blocks (8)
NHT = H // 128   # h tiles (32)
DC = D // 512    # output d chunks (2)

# ---- expert-parallel kernel geometry ----
EP_CAP = 2176                  # token capacity per expert (17 tiles of 128)
EP_NT = EP_CAP // 128          # 17
# All chunk widths >= 256 so fc1 matmul dur >= LDWEIGHTS dur (stays hidden).
# A small first chunk shrinks the x-chunk-0 transfer gating the first matmul.
EP_CHUNKS = [(0, 256), (256, 512), (768, 512), (1280, 512), (1792, 384)]
W1SL = 16                      # w1 load slices (2 h-tiles each)


def build_nc_ep():
    """Expert-parallel dense FFN: one expert per core, weights resident."""
    nc = bacc.Bacc("TRN2", target_bir_lowering=False, debug=False,
                   num_devices=NCORES)
    # xh is packed per chunk ([ds, tok] blocks) and w1d per 512-h group so
    # every DMA slice is contiguous per partition (8KB descriptors; a
    # last-dim slice of [128, DS, H] would issue 1KB descriptors and run
    # ~8x slower, gating the kernel start).
    xh = nc.dram_tensor("xh", [128, DS * EP_CAP], BF, kind="ExternalInput")
    w1d = nc.dram_tensor("w1d", [128, W1SL, DS, H // W1SL], BF,
                         kind="ExternalInput")
    w2d = nc.dram_tensor("w2d", [128, NHT, D], BF, kind="ExternalInput")
    b1d = nc.dram_tensor("b1d", [128, NHT], F32, kind="ExternalInput")
    gd = nc.dram_tensor("gd", [128, EP_NT], F32, kind="ExternalInput")
    outd = nc.dram_tensor("outd", [EP_CAP, D], F32, kind="ExternalOutput")

    with tile.TileContext(nc) as tc, ExitStack() as ctx:
        const = ctx.enter_context(tc.tile_pool(name="const", bufs=1))
        xp = ctx.enter_context(tc.tile_pool(name="xp", bufs=2))
        hp = ctx.enter_context(tc.tile_pool(name="hp", bufs=1))
        yp = ctx.enter_context(tc.tile_pool(name="yp", bufs=4))
        ps1 = ctx.enter_context(tc.tile_pool(name="ps1", bufs=3, space="PSUM"))
        ps2 = ctx.enter_context(tc.tile_pool(name="ps2", bufs=4, space="PSUM"))
        wps = ctx.enter_context(tc.tile_pool(name="wps", bufs=1, space="PSUM"))

        # Startup DMA plan — everything bulk goes on the SYNC queue, in
        # consumption order. Measured queue behavior: a single queue alone
        # sustains ~450GB/s, but two concurrent queues split by DESCRIPTOR
        # count, so a big-descriptor stream on one queue starves small
        # descriptors on the other (observed 400 vs 50 GB/s). Also, a
        # dma_start HOLDS its engine's FIFO while waiting for a ring slot,
        # so bulk loads must not sit on the Activation FIFO ahead of the
        # gelus. Hence:
        #   sync: x0 | w1 as 8 slice dmas (land every ~2.4us, fc1 consumes
        #         one per ~6.9us) | w2 | gates | x prefetches (chunk loop)
        #   act:  b1 (tiny) | y writebacks
        # Dependencies are full-completion per dma instruction, and a tile
        # written by k dmas blocks readers until all k land — hence one
        # tile per w1 slice.
        b1sb = const.tile([128, NHT], F32)
        nc.scalar.dma_start(out=b1sb[:], in_=b1d[:, :])
        xt0 = xp.tile([128, DS, 256], BF, tag="xt")
        nc.sync.dma_start(out=xt0[:], in_=xh[:, 0:DS * 256])
        w1s = []
        for i in range(W1SL):
            t = const.tile([128, DS, H // W1SL], BF, tag=f"w1_{i}")
            nc.sync.dma_start(out=t[:], in_=w1d[:, i, :, :])
            w1s.append(t)
        w2sb = const.tile([128, NHT, D], BF)
        nc.sync.dma_start(out=w2sb[:], in_=w2d[:, :, :])
        gsb = const.tile([128, EP_NT], F32)
        nc.sync.dma_start(out=gsb[:], in_=gd[:, :])

        # Warm the PE's HAM clock gate during the startup DMA window: memset
        # a scratch tile (DVE, no DMA dependency) and run ~3.6us of dummy
        # N=128 matmuls on it. They issue right after the preamble and end
        # as x0/w1_0 land, so the real stream starts at 2.4 GHz.
        wsrc = const.tile([128, 128], BF)
        nc.vector.memset(wsrc[:], 0.0)
        warm = wps.tile([128, 128], F32, tag="warm")
        for _ in range(36):
            nc.tensor.matmul(warm[:], lhsT=wsrc[:, :], rhs=wsrc[:, :],
                             start=True, stop=True)

        for ci, (t0, cw) in enumerate(EP_CHUNKS):
            if ci == 0:
                xt = xt0
            else:
                xt = xp.tile([128, DS, cw], BF, tag="xt")
                nc.sync.dma_start(out=xt[:],
                                  in_=xh[:, DS * t0:DS * (t0 + cw)])
            # ---------------- fc1: hT[h, tok] = gelu(w1 @ x + b1) --------
            hT = hp.tile([128, NHT, cw], BF, tag="hT")
            for ht in range(NHT):
                p1 = ps1.tile([128, cw], F32, tag="p1")
                for ds in range(DS):
                    nc.tensor.matmul(
                        p1[:],
                        lhsT=w1s[ht // 2][:, ds,
                                          (ht % 2) * 128:(ht % 2 + 1) * 128],
                        rhs=xt[:, ds, :],
                        start=(ds == 0),
                        stop=(ds == DS - 1),
                    )
                nc.scalar.activation(
                    hT[:, ht, :], p1[:], AF.Gelu_apprx_tanh,
                    bias=b1sb[:, ht:ht + 1],
                )
            # ------------ fc2: y[tok, d] = g * (hT.T @ w2)  (b2 on host) --
            for tt in range(cw // 128):
                gt = t0 // 128 + tt
                y = yp.tile([128, D], F32, tag="y")
                for dc in range(DC):
                    p2 = ps2.tile([128, 512], F32, tag="p2")
                    for ht in range(NHT):
                        nc.tensor.matmul(
                            p2[:],
                            lhsT=hT[:, ht, tt * 128:(tt + 1) * 128],
                            rhs=w2sb[:, ht, dc * 512:(dc + 1) * 512],
                            start=(ht == 0),
                            stop=(ht == NHT - 1),
                        )
                    nc.vector.tensor_scalar_mul(
                        y[:, dc * 512:(dc + 1) * 512], p2[:],
                        gsb[:, gt:gt + 1])
                nc.scalar.dma_start(
                    out=outd[t0 + tt * 128:t0 + (tt + 1) * 128, :],
                    in_=y[:])
    nc.compile()
    return nc


# ======================= dense fallback (token-parallel) ====================
TLOC = 1024      # tokens per core
CHUNK = 512      # tokens per hT block
NCH = TLOC // CHUNK
TT = CHUNK // 128        # token tiles per chunk (4)
W1G = H // 512           # 8 w1 DMA groups per expert, each [128, DS, 512]
HQ = 4                   # w2 h-quarters, each 8 h-tiles


def build_nc():
    nc = bacc.Bacc("TRN2", target_bir_lowering=False, debug=False,
                   num_devices=NCORES)
    xh = nc.dram_tensor("xh", [128, DS, TLOC], FR, kind="ExternalInput")
    w1h = nc.dram_tensor("w1h", [E, W1G, 128, DS, 512], FR, kind="ExternalInput")
    w2h = nc.dram_tensor("w2h", [E, DC, HQ, 128, 8, 512], FR, kind="ExternalInput")
    b1h = nc.dram_tensor("b1h", [128, E, NHT], F32, kind="ExternalInput")
    b2h = nc.dram_tensor("b2h", [E, D], FR, kind="ExternalInput")
    # host-computed gates: gh[p, tt_global, e] (token t = tt_global*128 + p)
    gh = nc.dram_tensor("gh", [128, TLOC // 128, E], F32, kind="ExternalInput")
    # transposed gates for the fc2-bias rank-1 term: ght[e, tok]
    ght = nc.dram_tensor("ght", [E, TLOC], FR, kind="ExternalInput")
    outd = nc.dram_tensor("outd", [NCH, 128, TT, DC, 512], F32,
                          kind="ExternalOutput")

    with tile.TileContext(nc) as tc, ExitStack() as ctx:
        const = ctx.enter_context(tc.tile_pool(name="const", bufs=1))
        hpool = ctx.enter_context(tc.tile_pool(name="hT", bufs=1))
        apool = ctx.enter_context(tc.tile_pool(name="oacc", bufs=2))
        w1p = ctx.enter_context(tc.tile_pool(name="w1", bufs=2))
        w2p = ctx.enter_context(tc.tile_pool(name="w2", bufs=2))
        ps1 = ctx.enter_context(tc.tile_pool(name="ps1", bufs=2, space="PSUM"))
        ps2 = ctx.enter_context(tc.tile_pool(name="ps2", bufs=6, space="PSUM"))

        # --- resident tensors ---
        xsb = const.tile([128, DS, TLOC], FR)
        nc.sync.dma_start(out=xsb[:], in_=xh[:, :, :])
        b1sb = const.tile([128, E, NHT], F32)
        nc.sync.dma_start(out=b1sb[:], in_=b1h[:, :, :])
        b2sb = const.tile([E, D], FR)
        nc.sync.dma_start(out=b2sb[:], in_=b2h[:, :])
        gsb = const.tile([128, TLOC // 128, E], F32)
        nc.sync.dma_start(out=gsb[:], in_=gh[:, :, :])
        gtsb = const.tile([E, TLOC], FR)
        nc.sync.dma_start(out=gtsb[:], in_=ght[:, :])

        for c in range(NCH):
            t0 = c * CHUNK
            # init oacc with the fc2 bias term: oacc[t, d] = sum_e g_e(t) b2_e(d)
            oacc = apool.tile([128, TT, DC, 512], F32)
            for tt in range(TT):
                for dc in range(DC):
                    pb = ps2.tile([128, 512], F32, name=f"pb{tt}_{dc}", tag="pst")
                    nc.tensor.matmul(
                        pb[:],
                        lhsT=gtsb[:, t0 + tt * 128: t0 + (tt + 1) * 128],
                        rhs=b2sb[:, dc * 512: (dc + 1) * 512],
                        start=True, stop=True,
                    )
                    nc.vector.tensor_copy(oacc[:, tt, dc, :], pb[:])

            for e in range(E):
                # ---------------- fc1: hT[h, tok] = gelu(w1 @ x + b1) --------
                hT = hpool.tile([128, NHT, CHUNK], FR)
                for wg in range(W1G):  # 8 groups x 4 h-tiles
                    w1t = w1p.tile([128, DS, 512], FR)
                    nc.sync.dma_start(out=w1t[:], in_=w1h[e, wg, :, :, :])
                    for hti in range(4):
                        ht = wg * 4 + hti
                        p1 = ps1.tile([128, 512], F32)
                        for ds in range(DS):
                            nc.tensor.matmul(
                                p1[:, :CHUNK],
                                lhsT=w1t[:, ds, hti * 128: (hti + 1) * 128],
                                rhs=xsb[:, ds, t0: t0 + CHUNK],
                                start=(ds == 0),
                                stop=(ds == DS - 1),
                            )
                        nc.scalar.activation(
                            hT[:, ht, :], p1[:, :CHUNK], AF.Gelu_apprx_tanh,
                            bias=b1sb[:, e, ht: ht + 1],
                        )
                # ---------------- fc2: out[tok, d] += g_e * (hT.T @ w2) ------
                for dc in range(DC):
                    pst = [ps2.tile([128, 512], F32, name=f"pst{_t}", tag="pst")
                           for _t in range(TT)]
                    for hq in range(HQ):
                        w2t = w2p.tile([128, 8, 512], FR)
                        nc.sync.dma_start(out=w2t[:], in_=w2h[e, dc, hq, :, :, :])
                        for hh in range(8):
                            ht = hq * 8 + hh
                            for tt in range(TT):
                                nc.tensor.matmul(
                                    pst[tt][:],
                                    lhsT=hT[:, ht, tt * 128: (tt + 1) * 128],
                                    rhs=w2t[:, hh, :],
                                    start=(hq == 0 and hh == 0),
                                    stop=(hq == HQ - 1 and hh == 7),
                                )
                    for tt in range(TT):
                        nc.vector.scalar_tensor_tensor(
                            out=oacc[:, tt, dc, :],
                            in0=pst[tt][:],
                            scalar=gsb[:, (t0 // 128) + tt, e: e + 1],
                            in1=oacc[:, tt, dc, :],
                            op0=OP.mult,
                            op1=OP.add,
                        )
            nc.sync.dma_start(out=outd[c, :, :, :, :], in_=oacc[:])
    nc.compile()
    return nc


_CACHE = {}


def _get_nc():
    if "nc" not in _CACHE:
        _CACHE["nc"] = build_nc()
    return _CACHE["nc"]


def _get_nc_ep():
    if "ncep" not in _CACHE:
        _CACHE["ncep"] = build_nc_ep()
    return _CACHE["ncep"]


def host_router(x, scale_embeddings, router_w, router_b, scale_idx):
    """Exact-fp32 router matching the reference's op order.

    Returns (gates [T, E] fp32, top2 idx [T, 2], top2 weights [T, 2]).
    """
    f = np.float32
    T = x.shape[0] * x.shape[1]
    xs = (x.astype(f, copy=False)
          + scale_embeddings[int(scale_idx)].astype(f, copy=False)[None, None, :])
    logits = (xs.reshape(T, D) @ router_w.astype(f, copy=False).T
              + router_b.astype(f, copy=False))                    # [T, E]
    # top-2 with jax.lax.top_k tie semantics (lowest index wins)
    neg = -logits
    idx = np.argsort(neg, axis=1, kind="stable")[:, :2]            # [T, 2]
    v = np.take_along_axis(logits, idx, axis=1)
    w = np.exp(v - v[:, :1])
    w = w / w.sum(axis=1, keepdims=True)
    w = w.astype(f)
    gates = np.zeros((T, E), f)
    np.put_along_axis(gates, idx, w, axis=1)
    return gates, idx, w


def make_in_maps_ep(x, scale_embeddings, router_w, router_b,
                    fc1_w, fc1_b, fc2_w, fc2_b, scale_idx):
    """Returns (in_maps, (B, S), flat) or None if any expert overflows CAP.

    flat[2T] are row indices into the concatenated [E*EP_CAP, D] device
    output: flat[:T] = slot-0 row for each token, flat[T:] = slot-1 row.
    """
    x = np.asarray(x, np.float32)
    B, S, _ = x.shape
    T = B * S
    assert x.shape[2] == D
    fc1_w = np.asarray(fc1_w)
    fc1_b = np.asarray(fc1_b)
    fc2_w = np.asarray(fc2_w)
    fc2_b = np.asarray(fc2_b)
    _, top_idx, top_w = host_router(
        x, np.asarray(scale_embeddings), np.asarray(router_w),
        np.asarray(router_b), np.asarray(scale_idx))
    e_all = np.concatenate([top_idx[:, 0], top_idx[:, 1]])         # [2T]
    t_all = np.concatenate([np.arange(T), np.arange(T)])
    w_all = np.concatenate([top_w[:, 0], top_w[:, 1]]).astype(np.float32)
    counts = np.bincount(e_all, minlength=E)
    if counts.max() > EP_CAP:
        return None
    order = np.argsort(e_all, kind="stable")
    off = np.zeros(E + 1, np.int64)
    off[1:] = np.cumsum(counts)
    tok_sorted = t_all[order]
    w_sorted = w_all[order]
    pos = np.empty(2 * T, np.int64)
    pos[order] = np.arange(2 * T)
    flat = e_all.astype(np.int64) * EP_CAP + (pos - off[e_all])

    # host-side fc2-bias term: bterm[t] = sum_s g_s(t) * b2[e_s(t)]
    b2f = fc2_b.astype(np.float32)
    bterm = (top_w[:, 0:1].astype(np.float32) * b2f[top_idx[:, 0]]
             + top_w[:, 1:2].astype(np.float32) * b2f[top_idx[:, 1]])

    xf = x.reshape(T, D)
    in_maps = []
    for e in range(E):
        idx_e = tok_sorted[off[e]:off[e + 1]]
        n_e = len(idx_e)
        xe = np.zeros((EP_CAP, D), np.float32)
        xe[:n_e] = xf[idx_e]
        # xh packed per chunk: block[p, ds, j] = xe[t0+j, ds*128 + p]
        blocks = []
        for (t0, cw) in EP_CHUNKS:
            blk = xe[t0:t0 + cw].reshape(cw, DS, 128).transpose(2, 1, 0)
            blocks.append(blk.reshape(128, DS * cw))
        xhh = np.concatenate(blocks, axis=1).astype(BF16)
        g = np.zeros(EP_CAP, np.float32)
        g[:n_e] = w_sorted[off[e]:off[e + 1]]
        gdd = np.ascontiguousarray(g.reshape(EP_NT, 128).T)        # [128, NT]
        # w1d[p, g, ds, j] = fc1_w[e, g*(H//W1SL)+j, ds*128+p]
        w1dd = (fc1_w[e].reshape(W1SL, H // W1SL, DS, 128)
                .transpose(3, 0, 2, 1).astype(BF16))
        # w2d[p, ht, d] = fc2_w[e, d, ht*128+p]
        w2dd = fc2_w[e].reshape(D, NHT, 128).transpose(2, 1, 0).astype(BF16)
        b1dd = np.ascontiguousarray(
            fc1_b[e].astype(np.float32).reshape(NHT, 128).T)       # [128, NHT]
        in_maps.append({
            "xh": xhh, "w1d": w1dd, "w2d": w2dd, "b1d": b1dd, "gd": gdd,
        })
    return in_maps, (B, S), flat, bterm


def _prep_shared(fc1_w, fc1_b, fc2_w, fc2_b):
    f = np.float32
    w1t = np.ascontiguousarray(fc1_w.transpose(0, 2, 1)).astype(f, copy=False)
    w1h = np.ascontiguousarray(
        w1t.reshape(E, DS, 128, W1G, 512).transpose(0, 3, 2, 1, 4))
    w2t = np.ascontiguousarray(fc2_w.transpose(0, 2, 1)).astype(f, copy=False)
    w2h = np.ascontiguousarray(
        w2t.reshape(E, HQ, 8, 128, DC, 512).transpose(0, 4, 1, 3, 2, 5))
    b1h = np.ascontiguousarray(
        fc1_b.astype(f, copy=False).reshape(E, NHT, 128).transpose(2, 0, 1))
    b2h = np.ascontiguousarray(fc2_b.astype(f, copy=False))
    return w1h, w2h, b1h, b2h


def make_in_maps(x, scale_embeddings, router_w, router_b,
                 fc1_w, fc1_b, fc2_w, fc2_b, scale_idx):
    x = np.asarray(x, np.float32)
    B, S, _ = x.shape
    T = B * S
    assert T == NCORES * TLOC and x.shape[2] == D
    w1h, w2h, b1h, b2h = _prep_shared(
        np.asarray(fc1_w), np.asarray(fc1_b),
        np.asarray(fc2_w), np.asarray(fc2_b))
    gates, _, _ = host_router(x, np.asarray(scale_embeddings),
                              np.asarray(router_w), np.asarray(router_b),
                              np.asarray(scale_idx))
    xf = x.reshape(T, D)
    in_maps = []
    for i in range(NCORES):
        xloc = xf[i * TLOC:(i + 1) * TLOC]                       # [TLOC, D]
        xT = np.ascontiguousarray(xloc.T)                        # [D, TLOC]
        xhh = np.ascontiguousarray(
            xT.reshape(DS, 128, TLOC).transpose(1, 0, 2))        # [128, DS, TLOC]
        gloc = gates[i * TLOC:(i + 1) * TLOC]                    # [TLOC, E]
        ghh = np.ascontiguousarray(
            gloc.reshape(TLOC // 128, 128, E).transpose(1, 0, 2))
        ght = np.ascontiguousarray(gloc.T)                       # [E, TLOC]
        in_maps.append({
            "xh": xhh, "w1h": w1h, "w2h": w2h, "b1h": b1h,
            "b2h": b2h, "gh": ghh, "ght": ght,
        })
    return in_maps, (B, S)


def kernel(x, scale_embeddings, router_w, router_b,
           fc1_w, fc1_b, fc2_w, fc2_b, scale_idx):
    args = (x, scale_embeddings, router_w, router_b,
            fc1_w, fc1_b, fc2_w, fc2_b, scale_idx)
    ep = make_in_maps_ep(*args)
    if ep is not None:
        in_maps, (B, S), flat, bterm = ep
        T = B * S
        nc = _get_nc_ep()
        res = run_bass_kernel_spmd(nc, in_maps, core_ids=list(range(NCORES)))
        Y = np.stack([res.results[i]["outd"] for i in range(NCORES)])
        Yf = Y.reshape(E * EP_CAP, D)
        out = Yf[flat[:T]] + Yf[flat[T:]] + bterm
        return out.reshape(B, S, D)
    # capacity overflow (practically impossible): dense fallback
    in_maps, (B, S) = make_in_maps(*args)
    nc = _get_nc()
    res = run_bass_kernel_spmd(nc, in_maps, core_ids=list(range(NCORES)))
    parts = []
    for i in range(NCORES):
        o = res.results[i]["outd"]                               # [NCH,128,TT,DC,512]
        parts.append(o.transpose(0, 2, 1, 3, 4).reshape(TLOC, D))
    return np.concatenate(parts, 0).reshape(B, S, D)


# revision 30
# speedup vs baseline: 1.0245x; 1.0245x over previous
"""MoE FFN (8 experts, top-2) Trainium2 Bass kernel.

Strategy: EXPERT-parallel across the 8 cores. Core e owns expert e's
weights, resident in SBUF as bf16 (w1 64KiB/partition + w2 64KiB/partition),
and processes every token routed to expert e (CAP=2176 padded slots; actual
max expert load for this input distribution is ~2120 of mean 2048). The tiny
router (0.06% of FLOPs) runs on host in exact fp32 (matching the reference's
op order so top-2 selection is stable); the host also gathers/transposes each
expert's tokens and scatter-gathers the outputs back, so the device kernel is
a pure dense FFN: hT = gelu_tanh(w1 @ x + b1) -> y = gate * (hT.T @ w2 + b2).

This beats token-parallel (capacity 384 per (core,expert) = 3072 slots/core,
1.5x padding) at 2176 slots/core (1.06x padding), removes all device-side
gathers/transposes/scatters, and streams weights exactly once (resident).

Everything stays in [feature, token] layout on chip: fc1 consumes w1T tiles
as stationary and xT as moving; fc2 consumes hT tiles as stationary and w2T
as moving, producing [token, d] PSUM tiles so the per-token gate is a
per-partition scalar multiply (one DVE op). b2 is added via a K=1 ones
matmul folded into the fc2 PSUM accumulation. Matmul operands are bf16
(fp32 PSUM accumulate): rel err ~5e-3, well under the 2e-2 gate.
"""

import numpy as np
from contextlib import ExitStack

import ml_dtypes

import concourse.bass as bass
import concourse.bacc as bacc
import concourse.tile as tile
from concourse import mybir
from concourse.bass_utils import run_bass_kernel_spmd

BF16 = ml_dtypes.bfloat16
FR = mybir.dt.float32r
BF = mybir.dt.bfloat16
F32 = mybir.dt.float32
AF = mybir.ActivationFunctionType
OP = mybir.AluOpType

NCORES = 8
E = 8            # experts
D = 1024         # model dim
H = 4096         # hidden dim
DS = D // 128    # d sub-blocks (8)
NHT = H // 128   # h tiles (32)
DC = D // 512    # output d chunks (2)

# ---- expert-parallel kernel geometry ----
EP_CAP = 2176                  # token capacity per expert (17 tiles of 128)
EP_NT = EP_CAP // 128          # 17
# All chunk widths >= 256 so fc1 matmul dur >= LDWEIGHTS dur (stays hidden).
# 512 first: fc1 chunk 0 must run ~55us so the w2 transfer (queued behind
# w1 on the sync queue) lands before fc2 chunk 0 needs it.
EP_CHUNKS = [(0, 512), (512, 512), (1024, 512), (1536, 384), (1920, 256)]
W1SL = 16                      # w1 load slices (2 h-tiles each)


def build_nc_ep():
    """Expert-parallel dense FFN: one expert per core, weights resident."""
    nc = bacc.Bacc("TRN2", target_bir_lowering=False, debug=False,
                   num_devices=NCORES)
    # xh is packed per chunk ([ds, tok] blocks) and w1d per 512-h group so
    # every DMA slice is contiguous per partition (8KB descriptors; a
    # last-dim slice of [128, DS, H] would issue 1KB descriptors and run
    # ~8x slower, gating the kernel start).
    xh = nc.dram_tensor("xh", [128, DS * EP_CAP], BF, kind="ExternalInput")
    w1d = nc.dram_tensor("w1d", [128, W1SL, DS, H // W1SL], BF,
                         kind="ExternalInput")
    w2d = nc.dram_tensor("w2d", [128, NHT, D], BF, kind="ExternalInput")
    b1d = nc.dram_tensor("b1d", [128, NHT], F32, kind="ExternalInput")
    gd = nc.dram_tensor("gd", [128, EP_NT], F32, kind="ExternalInput")
    outd = nc.dram_tensor("outd", [EP_CAP, D], F32, kind="ExternalOutput")

    with tile.TileContext(nc) as tc, ExitStack() as ctx:
        const = ctx.enter_context(tc.tile_pool(name="const", bufs=1))
        xp = ctx.enter_context(tc.tile_pool(name="xp", bufs=2))
        hp = ctx.enter_context(tc.tile_pool(name="hp", bufs=1))
        yp = ctx.enter_context(tc.tile_pool(name="yp", bufs=4))
        ps1 = ctx.enter_context(tc.tile_pool(name="ps1", bufs=3, space="PSUM"))
        ps2 = ctx.enter_context(tc.tile_pool(name="ps2", bufs=4, space="PSUM"))
        wps = ctx.enter_context(tc.tile_pool(name="wps", bufs=1, space="PSUM"))

        # Startup DMA plan — everything bulk goes on the SYNC queue, in
        # consumption order. Measured queue behavior: a single queue alone
        # sustains ~450GB/s, but two concurrent queues split by DESCRIPTOR
        # count, so a big-descriptor stream on one queue starves small
        # descriptors on the other (observed 400 vs 50 GB/s). Also, a
        # dma_start HOLDS its engine's FIFO while waiting for a ring slot,
        # so bulk loads must not sit on the Activation FIFO ahead of the
        # gelus. Hence:
        #   sync: x0 | w1 as 8 slice dmas (land every ~2.4us, fc1 consumes
        #         one per ~6.9us) | w2 | gates | x prefetches (chunk loop)
        #   act:  b1 (tiny) | y writebacks
        # Dependencies are full-completion per dma instruction, and a tile
        # written by k dmas blocks readers until all k land — hence one
        # tile per w1 slice.
        b1sb = const.tile([128, NHT], F32)
        nc.scalar.dma_start(out=b1sb[:], in_=b1d[:, :])
        xt0 = xp.tile([128, DS, 512], BF, tag="xt")
        nc.sync.dma_start(out=xt0[:], in_=xh[:, 0:DS * 512])
        w1s = []
        for i in range(W1SL):
            t = const.tile([128, DS, H // W1SL], BF, tag=f"w1_{i}")
            nc.sync.dma_start(out=t[:], in_=w1d[:, i, :, :])
            w1s.append(t)
        w2sb = const.tile([128, NHT, D], BF)
        nc.sync.dma_start(out=w2sb[:], in_=w2d[:, :, :])
        gsb = const.tile([128, EP_NT], F32)
        nc.sync.dma_start(out=gsb[:], in_=gd[:, :])

        # Warm the PE's HAM clock gate during the startup DMA window: memset
        # a scratch tile (DVE, no DMA dependency) and run ~3.6us of dummy
        # N=128 matmuls on it. They issue right after the preamble and end
        # as x0/w1_0 land, so the real stream starts at 2.4 GHz.
        wsrc = const.tile([128, 128], BF)
        nc.vector.memset(wsrc[:], 0.0)
        warm = wps.tile([128, 128], F32, tag="warm")
        for _ in range(36):
            nc.tensor.matmul(warm[:], lhsT=wsrc[:, :], rhs=wsrc[:, :],
                             start=True, stop=True)

        for ci, (t0, cw) in enumerate(EP_CHUNKS):
            if ci == 0:
                xt = xt0
            else:
                xt = xp.tile([128, DS, cw], BF, tag="xt")
                nc.sync.dma_start(out=xt[:],
                                  in_=xh[:, DS * t0:DS * (t0 + cw)])
            # ---------------- fc1: hT[h, tok] = gelu(w1 @ x + b1) --------
            hT = hp.tile([128, NHT, cw], BF, tag="hT")
            for ht in range(NHT):
                p1 = ps1.tile([128, cw], F32, tag="p1")
                for ds in range(DS):
                    nc.tensor.matmul(
                        p1[:],
                        lhsT=w1s[ht // 2][:, ds,
                                          (ht % 2) * 128:(ht % 2 + 1) * 128],
                        rhs=xt[:, ds, :],
                        start=(ds == 0),
                        stop=(ds == DS - 1),
                    )
                nc.scalar.activation(
                    hT[:, ht, :], p1[:], AF.Gelu_apprx_tanh,
                    bias=b1sb[:, ht:ht + 1],
                )
            # ------------ fc2: y[tok, d] = g * (hT.T @ w2)  (b2 on host) --
            for tt in range(cw // 128):
                gt = t0 // 128 + tt
                y = yp.tile([128, D], F32, tag="y")
                for dc in range(DC):
                    p2 = ps2.tile([128, 512], F32, tag="p2")
                    for ht in range(NHT):
                        nc.tensor.matmul(
                            p2[:],
                            lhsT=hT[:, ht, tt * 128:(tt + 1) * 128],
                            rhs=w2sb[:, ht, dc * 512:(dc + 1) * 512],
                            start=(ht == 0),
                            stop=(ht == NHT - 1),
                        )
                    nc.vector.tensor_scalar_mul(
                        y[:, dc * 512:(dc + 1) * 512], p2[:],
                        gsb[:, gt:gt + 1])
                nc.scalar.dma_start(
                    out=outd[t0 + tt * 128:t0 + (tt + 1) * 128, :],
                    in_=y[:])
    nc.compile()
    return nc


# ======================= dense fallback (token-parallel) ====================
TLOC = 1024      # tokens per core
CHUNK = 512      # tokens per hT block
NCH = TLOC // CHUNK
TT = CHUNK // 128        # token tiles per chunk (4)
W1G = H // 512           # 8 w1 DMA groups per expert, each [128, DS, 512]
HQ = 4                   # w2 h-quarters, each 8 h-tiles


def build_nc():
    nc = bacc.Bacc("TRN2", target_bir_lowering=False, debug=False,
                   num_devices=NCORES)
    xh = nc.dram_tensor("xh", [128, DS, TLOC], FR, kind="ExternalInput")
    w1h = nc.dram_tensor("w1h", [E, W1G, 128, DS, 512], FR, kind="ExternalInput")
    w2h = nc.dram_tensor("w2h", [E, DC, HQ, 128, 8, 512], FR, kind="ExternalInput")
    b1h = nc.dram_tensor("b1h", [128, E, NHT], F32, kind="ExternalInput")
    b2h = nc.dram_tensor("b2h", [E, D], FR, kind="ExternalInput")
    # host-computed gates: gh[p, tt_global, e] (token t = tt_global*128 + p)
    gh = nc.dram_tensor("gh", [128, TLOC // 128, E], F32, kind="ExternalInput")
    # transposed gates for the fc2-bias rank-1 term: ght[e, tok]
    ght = nc.dram_tensor("ght", [E, TLOC], FR, kind="ExternalInput")
    outd = nc.dram_tensor("outd", [NCH, 128, TT, DC, 512], F32,
                          kind="ExternalOutput")

    with tile.TileContext(nc) as tc, ExitStack() as ctx:
        const = ctx.enter_context(tc.tile_pool(name="const", bufs=1))
        hpool = ctx.enter_context(tc.tile_pool(name="hT", bufs=1))
        apool = ctx.enter_context(tc.tile_pool(name="oacc", bufs=2))
        w1p = ctx.enter_context(tc.tile_pool(name="w1", bufs=2))
        w2p = ctx.enter_context(tc.tile_pool(name="w2", bufs=2))
        ps1 = ctx.enter_context(tc.tile_pool(name="ps1", bufs=2, space="PSUM"))
        ps2 = ctx.enter_context(tc.tile_pool(name="ps2", bufs=6, space="PSUM"))

        # --- resident tensors ---
        xsb = const.tile([128, DS, TLOC], FR)
        nc.sync.dma_start(out=xsb[:], in_=xh[:, :, :])
        b1sb = const.tile([128, E, NHT], F32)
        nc.sync.dma_start(out=b1sb[:], in_=b1h[:, :, :])
        b2sb = const.tile([E, D], FR)
        nc.sync.dma_start(out=b2sb[:], in_=b2h[:, :])
        gsb = const.tile([128, TLOC // 128, E], F32)
        nc.sync.dma_start(out=gsb[:], in_=gh[:, :, :])
        gtsb = const.tile([E, TLOC], FR)
        nc.sync.dma_start(out=gtsb[:], in_=ght[:, :])

        for c in range(NCH):
            t0 = c * CHUNK
            # init oacc with the fc2 bias term: oacc[t, d] = sum_e g_e(t) b2_e(d)
            oacc = apool.tile([128, TT, DC, 512], F32)
            for tt in range(TT):
                for dc in range(DC):
                    pb = ps2.tile([128, 512], F32, name=f"pb{tt}_{dc}", tag="pst")
                    nc.tensor.matmul(
                        pb[:],
                        lhsT=gtsb[:, t0 + tt * 128: t0 + (tt + 1) * 128],
                        rhs=b2sb[:, dc * 512: (dc + 1) * 512],
                        start=True, stop=True,
                    )
                    nc.vector.tensor_copy(oacc[:, tt, dc, :], pb[:])

            for e in range(E):
                # ---------------- fc1: hT[h, tok] = gelu(w1 @ x + b1) --------
                hT = hpool.tile([128, NHT, CHUNK], FR)
                for wg in range(W1G):  # 8 groups x 4 h-tiles
                    w1t = w1p.tile([128, DS, 512], FR)
                    nc.sync.dma_start(out=w1t[:], in_=w1h[e, wg, :, :, :])
                    for hti in range(4):
                        ht = wg * 4 + hti
                        p1 = ps1.tile([128, 512], F32)
                        for ds in range(DS):
                            nc.tensor.matmul(
                                p1[:, :CHUNK],
                                lhsT=w1t[:, ds, hti * 128: (hti + 1) * 128],
                                rhs=xsb[:, ds, t0: t0 + CHUNK],
                                start=(ds == 0),
                                stop=(ds == DS - 1),
                            )
                        nc.scalar.activation(
                            hT[:, ht, :], p1[:, :CHUNK], AF.Gelu_apprx_tanh,
                            bias=b1sb[:, e, ht: ht + 1],
                        )
                # ---------------- fc2: out[tok, d] += g_e * (hT.T @ w2) ------
                for dc in range(DC):
                    pst = [ps2.tile([128, 512], F32, name=f"pst{_t}", tag="pst")
                           for _t in range(TT)]
                    for hq in range(HQ):
                        w2t = w2p.tile([128, 8, 512], FR)
                        nc.sync.dma_start(out=w2t[:], in_=w2h[e, dc, hq, :, :, :])
                        for hh in range(8):
                            ht = hq * 8 + hh
                            for tt in range(TT):
                                nc.tensor.matmul(
                                    pst[tt][:],
                                    lhsT=hT[:, ht, tt * 128: (tt + 1) * 128],
                                    rhs=w2t[:, hh, :],
                                    start=(hq == 0 and hh == 0),
                                    stop=(hq == HQ - 1 and hh == 7),
                                )
                    for tt in range(TT):
                        nc.vector.scalar_tensor_tensor(
                            out=oacc[:, tt, dc, :],
                            in0=pst[tt][:],
                            scalar=gsb[:, (t0 // 128) + tt, e: e + 1],
                            in1=oacc[:, tt, dc, :],
                            op0=OP.mult,
                            op1=OP.add,
                        )
            nc.sync.dma_start(out=outd[c, :, :, :, :], in_=oacc[:])
    nc.compile()
    return nc


_CACHE = {}


def _get_nc():
    if "nc" not in _CACHE:
        _CACHE["nc"] = build_nc()
    return _CACHE["nc"]


def _get_nc_ep():
    if "ncep" not in _CACHE:
        _CACHE["ncep"] = build_nc_ep()
    return _CACHE["ncep"]


def host_router(x, scale_embeddings, router_w, router_b, scale_idx):
    """Exact-fp32 router matching the reference's op order.

    Returns (gates [T, E] fp32, top2 idx [T, 2], top2 weights [T, 2]).
    """
    f = np.float32
    T = x.shape[0] * x.shape[1]
    xs = (x.astype(f, copy=False)
          + scale_embeddings[int(scale_idx)].astype(f, copy=False)[None, None, :])
    logits = (xs.reshape(T, D) @ router_w.astype(f, copy=False).T
              + router_b.astype(f, copy=False))                    # [T, E]
    # top-2 with jax.lax.top_k tie semantics (lowest index wins)
    neg = -logits
    idx = np.argsort(neg, axis=1, kind="stable")[:, :2]            # [T, 2]
    v = np.take_along_axis(logits, idx, axis=1)
    w = np.exp(v - v[:, :1])
    w = w / w.sum(axis=1, keepdims=True)
    w = w.astype(f)
    gates = np.zeros((T, E), f)
    np.put_along_axis(gates, idx, w, axis=1)
    return gates, idx, w


def make_in_maps_ep(x, scale_embeddings, router_w, router_b,
                    fc1_w, fc1_b, fc2_w, fc2_b, scale_idx):
    """Returns (in_maps, (B, S), flat) or None if any expert overflows CAP.

    flat[2T] are row indices into the concatenated [E*EP_CAP, D] device
    output: flat[:T] = slot-0 row for each token, flat[T:] = slot-1 row.
    """
    x = np.asarray(x, np.float32)
    B, S, _ = x.shape
    T = B * S
    assert x.shape[2] == D
    fc1_w = np.asarray(fc1_w)
    fc1_b = np.asarray(fc1_b)
    fc2_w = np.asarray(fc2_w)
    fc2_b = np.asarray(fc2_b)
    _, top_idx, top_w = host_router(
        x, np.asarray(scale_embeddings), np.asarray(router_w),
        np.asarray(router_b), np.asarray(scale_idx))
    e_all = np.concatenate([top_idx[:, 0], top_idx[:, 1]])         # [2T]
    t_all = np.concatenate([np.arange(T), np.arange(T)])
    w_all = np.concatenate([top_w[:, 0], top_w[:, 1]]).astype(np.float32)
    counts = np.bincount(e_all, minlength=E)
    if counts.max() > EP_CAP:
        return None
    order = np.argsort(e_all, kind="stable")
    off = np.zeros(E + 1, np.int64)
    off[1:] = np.cumsum(counts)
    tok_sorted = t_all[order]
    w_sorted = w_all[order]
    pos = np.empty(2 * T, np.int64)
    pos[order] = np.arange(2 * T)
    flat = e_all.astype(np.int64) * EP_CAP + (pos - off[e_all])

    # host-side fc2-bias term: bterm[t] = sum_s g_s(t) * b2[e_s(t)]
    b2f = fc2_b.astype(np.float32)
    bterm = (top_w[:, 0:1].astype(np.float32) * b2f[top_idx[:, 0]]
             + top_w[:, 1:2].astype(np.float32) * b2f[top_idx[:, 1]])

    xf = x.reshape(T, D)
    in_maps = []
    for e in range(E):
        idx_e = tok_sorted[off[e]:off[e + 1]]
        n_e = len(idx_e)
        xe = np.zeros((EP_CAP, D), np.float32)
        xe[:n_e] = xf[idx_e]
        # xh packed per chunk: block[p, ds, j] = xe[t0+j, ds*128 + p]
        blocks = []
        for (t0, cw) in EP_CHUNKS:
            blk = xe[t0:t0 + cw].reshape(cw, DS, 128).transpose(2, 1, 0)
            blocks.append(blk.reshape(128, DS * cw))
        xhh = np.concatenate(blocks, axis=1).astype(BF16)
        g = np.zeros(EP_CAP, np.float32)
        g[:n_e] = w_sorted[off[e]:off[e + 1]]
        gdd = np.ascontiguousarray(g.reshape(EP_NT, 128).T)        # [128, NT]
        # w1d[p, g, ds, j] = fc1_w[e, g*(H//W1SL)+j, ds*128+p]
        w1dd = (fc1_w[e].reshape(W1SL, H // W1SL, DS, 128)
                .transpose(3, 0, 2, 1).astype(BF16))
        # w2d[p, ht, d] = fc2_w[e, d, ht*128+p]
        w2dd = fc2_w[e].reshape(D, NHT, 128).transpose(2, 1, 0).astype(BF16)
        b1dd = np.ascontiguousarray(
            fc1_b[e].astype(np.float32).reshape(NHT, 128).T)       # [128, NHT]
        in_maps.append({
            "xh": xhh, "w1d": w1dd, "w2d": w2dd, "b1d": b1dd, "gd": gdd,
        })
    return in_maps, (B, S), flat, bterm


def _prep_shared(fc1_w, fc1_b, fc2_w, fc2_b):
    f = np.float32
    w1t = np.ascontiguousarray(fc1_w.transpose(0, 2, 1)).astype(f, copy=False)
    w1h = np.ascontiguousarray(
        w1t.reshape(E, DS, 128, W1G, 512).transpose(0, 3, 2, 1, 4))
    w2t = np.ascontiguousarray(fc2_w.transpose(0, 2, 1)).astype(f, copy=False)
    w2h = np.ascontiguousarray(
        w2t.reshape(E, HQ, 8, 128, DC, 512).transpose(0, 4, 1, 3, 2, 5))
    b1h = np.ascontiguousarray(
        fc1_b.astype(f, copy=False).reshape(E, NHT, 128).transpose(2, 0, 1))
    b2h = np.ascontiguousarray(fc2_b.astype(f, copy=False))
    return w1h, w2h, b1h, b2h


def make_in_maps(x, scale_embeddings, router_w, router_b,
                 fc1_w, fc1_b, fc2_w, fc2_b, scale_idx):
    x = np.asarray(x, np.float32)
    B, S, _ = x.shape
    T = B * S
    assert T == NCORES * TLOC and x.shape[2] == D
    w1h, w2h, b1h, b2h = _prep_shared(
        np.asarray(fc1_w), np.asarray(fc1_b),
        np.asarray(fc2_w), np.asarray(fc2_b))
    gates, _, _ = host_router(x, np.asarray(scale_embeddings),
                              np.asarray(router_w), np.asarray(router_b),
                              np.asarray(scale_idx))
    xf = x.reshape(T, D)
    in_maps = []
    for i in range(NCORES):
        xloc = xf[i * TLOC:(i + 1) * TLOC]                       # [TLOC, D]
        xT = np.ascontiguousarray(xloc.T)                        # [D, TLOC]
        xhh = np.ascontiguousarray(
            xT.reshape(DS, 128, TLOC).transpose(1, 0, 2))        # [128, DS, TLOC]
        gloc = gates[i * TLOC:(i + 1) * TLOC]                    # [TLOC, E]
        ghh = np.ascontiguousarray(
            gloc.reshape(TLOC // 128, 128, E).transpose(1, 0, 2))
        ght = np.ascontiguousarray(gloc.T)                       # [E, TLOC]
        in_maps.append({
            "xh": xhh, "w1h": w1h, "w2h": w2h, "b1h": b1h,
            "b2h": b2h, "gh": ghh, "ght": ght,
        })
    return in_maps, (B, S)


def kernel(x, scale_embeddings, router_w, router_b,
           fc1_w, fc1_b, fc2_w, fc2_b, scale_idx):
    args = (x, scale_embeddings, router_w, router_b,
            fc1_w, fc1_b, fc2_w, fc2_b, scale_idx)
    ep = make_in_maps_ep(*args)
    if ep is not None:
        in_maps, (B, S), flat, bterm = ep
        T = B * S
        nc = _get_nc_ep()
        res = run_bass_kernel_spmd(nc, in_maps, core_ids=list(range(NCORES)))
        Y = np.stack([res.results[i]["outd"] for i in range(NCORES)])
        Yf = Y.reshape(E * EP_CAP, D)
        out = Yf[flat[:T]] + Yf[flat[T:]] + bterm
        return out.reshape(B, S, D)
    # capacity overflow (practically impossible): dense fallback
    in_maps, (B, S) = make_in_maps(*args)
    nc = _get_nc()
    res = run_bass_kernel_spmd(nc, in_maps, core_ids=list(range(NCORES)))
    parts = []
    for i in range(NCORES):
        o = res.results[i]["outd"]                               # [NCH,128,TT,DC,512]
        parts.append(o.transpose(0, 2, 1, 3, 4).reshape(TLOC, D))
    return np.concatenate(parts, 0).reshape(B, S, D)


# revision 32
# speedup vs baseline: 1.0255x; 1.0010x over previous
"""MoE FFN (8 experts, top-2) Trainium2 Bass kernel.

Strategy: EXPERT-parallel across the 8 cores. Core e owns expert e's
weights, resident in SBUF as bf16 (w1 64KiB/partition + w2 64KiB/partition),
and processes every token routed to expert e (CAP=2176 padded slots; actual
max expert load for this input distribution is ~2120 of mean 2048). The tiny
router (0.06% of FLOPs) runs on host in exact fp32 (matching the reference's
op order so top-2 selection is stable); the host also gathers/transposes each
expert's tokens and scatter-gathers the outputs back, so the device kernel is
a pure dense FFN: hT = gelu_tanh(w1 @ x + b1) -> y = gate * (hT.T @ w2 + b2).

This beats token-parallel (capacity 384 per (core,expert) = 3072 slots/core,
1.5x padding) at 2176 slots/core (1.06x padding), removes all device-side
gathers/transposes/scatters, and streams weights exactly once (resident).

Everything stays in [feature, token] layout on chip: fc1 consumes w1T tiles
as stationary and xT as moving; fc2 consumes hT tiles as stationary and w2T
as moving, producing [token, d] PSUM tiles so the per-token gate is a
per-partition scalar multiply (one DVE op). b2 is added via a K=1 ones
matmul folded into the fc2 PSUM accumulation. Matmul operands are bf16
(fp32 PSUM accumulate): rel err ~5e-3, well under the 2e-2 gate.
"""

import numpy as np
from contextlib import ExitStack

import ml_dtypes

import concourse.bass as bass
import concourse.bacc as bacc
import concourse.tile as tile
from concourse import mybir
from concourse.bass_utils import run_bass_kernel_spmd

BF16 = ml_dtypes.bfloat16
FR = mybir.dt.float32r
BF = mybir.dt.bfloat16
F32 = mybir.dt.float32
AF = mybir.ActivationFunctionType
OP = mybir.AluOpType

NCORES = 8
E = 8            # experts
D = 1024         # model dim
H = 4096         # hidden dim
DS = D // 128    # d sub-blocks (8)
NHT = H // 128   # h tiles (32)
DC = D // 512    # output d chunks (2)

# ---- expert-parallel kernel geometry ----
EP_CAP = 2176                  # token capacity per expert (17 tiles of 128)
EP_NT = EP_CAP // 128          # 17
# All chunk widths >= 256 so fc1 matmul dur >= LDWEIGHTS dur (stays hidden).
# 512 first: fc1 chunk 0 must run ~55us so the w2 transfer (queued behind
# w1 on the sync queue) lands before fc2 chunk 0 needs it.
EP_CHUNKS = [(0, 512), (512, 512), (1024, 512), (1536, 384), (1920, 256)]
W1SL = 16                      # w1 load slices (2 h-tiles each)


def build_nc_ep():
    """Expert-parallel dense FFN: one expert per core, weights resident."""
    nc = bacc.Bacc("TRN2", target_bir_lowering=False, debug=False,
                   num_devices=NCORES)
    # xh is packed per chunk ([ds, tok] blocks) and w1d per 512-h group so
    # every DMA slice is contiguous per partition (8KB descriptors; a
    # last-dim slice of [128, DS, H] would issue 1KB descriptors and run
    # ~8x slower, gating the kernel start).
    xh = nc.dram_tensor("xh", [128, DS * EP_CAP], BF, kind="ExternalInput")
    w1d = nc.dram_tensor("w1d", [128, W1SL, DS, H // W1SL], BF,
                         kind="ExternalInput")
    w2d = nc.dram_tensor("w2d", [128, NHT, D], BF, kind="ExternalInput")
    b1d = nc.dram_tensor("b1d", [128, NHT], F32, kind="ExternalInput")
    gd = nc.dram_tensor("gd", [128, EP_NT], F32, kind="ExternalInput")
    outd = nc.dram_tensor("outd", [EP_CAP, D], F32, kind="ExternalOutput")

    with tile.TileContext(nc) as tc, ExitStack() as ctx:
        const = ctx.enter_context(tc.tile_pool(name="const", bufs=1))
        xp = ctx.enter_context(tc.tile_pool(name="xp", bufs=2))
        hp = ctx.enter_context(tc.tile_pool(name="hp", bufs=1))
        yp = ctx.enter_context(tc.tile_pool(name="yp", bufs=4))
        ps1 = ctx.enter_context(tc.tile_pool(name="ps1", bufs=3, space="PSUM"))
        ps2 = ctx.enter_context(tc.tile_pool(name="ps2", bufs=4, space="PSUM"))
        wps = ctx.enter_context(tc.tile_pool(name="wps", bufs=1, space="PSUM"))

        # Startup DMA plan — everything bulk goes on the SYNC queue, in
        # consumption order. Measured queue behavior: a single queue alone
        # sustains ~450GB/s, but two concurrent queues split by DESCRIPTOR
        # count, so a big-descriptor stream on one queue starves small
        # descriptors on the other (observed 400 vs 50 GB/s). Also, a
        # dma_start HOLDS its engine's FIFO while waiting for a ring slot,
        # so bulk loads must not sit on the Activation FIFO ahead of the
        # gelus. Hence:
        #   sync: x0 | w1 as 8 slice dmas (land every ~2.4us, fc1 consumes
        #         one per ~6.9us) | w2 | gates | x prefetches (chunk loop)
        #   act:  b1 (tiny) | y writebacks
        # Dependencies are full-completion per dma instruction, and a tile
        # written by k dmas blocks readers until all k land — hence one
        # tile per w1 slice.
        b1sb = const.tile([128, NHT], F32)
        nc.scalar.dma_start(out=b1sb[:], in_=b1d[:, :])
        xt0 = xp.tile([128, DS, 512], BF, tag="xt")
        nc.sync.dma_start(out=xt0[:], in_=xh[:, 0:DS * 512])
        w1s = []
        for i in range(W1SL):
            t = const.tile([128, DS, H // W1SL], BF, tag=f"w1_{i}")
            nc.sync.dma_start(out=t[:], in_=w1d[:, i, :, :])
            w1s.append(t)
        w2sb = const.tile([128, NHT, D], BF)
        nc.sync.dma_start(out=w2sb[:], in_=w2d[:, :, :])
        gsb = const.tile([128, EP_NT], F32)
        nc.sync.dma_start(out=gsb[:], in_=gd[:, :])

        # Warm the PE's HAM clock gate during the startup DMA window: memset
        # a scratch tile (DVE, no DMA dependency) and run dummy N=128
        # matmuls on it. They issue right after the preamble and bridge the
        # ~7.5us until x0 + the first w1 slice land (~36 at the cold clock,
        # then ~56ns each warm), so the real stream starts at 2.4 GHz with
        # no HAM re-throttle.
        wsrc = const.tile([128, 128], BF)
        nc.vector.memset(wsrc[:], 0.0)
        warm = wps.tile([128, 128], F32, tag="warm")
        for _ in range(100):
            nc.tensor.matmul(warm[:], lhsT=wsrc[:, :], rhs=wsrc[:, :],
                             start=True, stop=True)

        for ci, (t0, cw) in enumerate(EP_CHUNKS):
            if ci == 0:
                xt = xt0
            else:
                xt = xp.tile([128, DS, cw], BF, tag="xt")
                nc.sync.dma_start(out=xt[:],
                                  in_=xh[:, DS * t0:DS * (t0 + cw)])
            # ---------------- fc1: hT[h, tok] = gelu(w1 @ x + b1) --------
            hT = hp.tile([128, NHT, cw], BF, tag="hT")
            for ht in range(NHT):
                p1 = ps1.tile([128, cw], F32, tag="p1")
                for ds in range(DS):
                    nc.tensor.matmul(
                        p1[:],
                        lhsT=w1s[ht // 2][:, ds,
                                          (ht % 2) * 128:(ht % 2 + 1) * 128],
                        rhs=xt[:, ds, :],
                        start=(ds == 0),
                        stop=(ds == DS - 1),
                    )
                nc.scalar.activation(
                    hT[:, ht, :], p1[:], AF.Gelu_apprx_tanh,
                    bias=b1sb[:, ht:ht + 1],
                )
            # ------------ fc2: y[tok, d] = g * (hT.T @ w2)  (b2 on host) --
            # y is written back per d-half so the dc=0 transfer overlaps the
            # dc=1 matmul group (shortens the post-stream drain).
            for tt in range(cw // 128):
                gt = t0 // 128 + tt
                y = yp.tile([128, D], F32, tag="y")
                for dc in range(DC):
                    p2 = ps2.tile([128, 512], F32, tag="p2")
                    for ht in range(NHT):
                        nc.tensor.matmul(
                            p2[:],
                            lhsT=hT[:, ht, tt * 128:(tt + 1) * 128],
                            rhs=w2sb[:, ht, dc * 512:(dc + 1) * 512],
                            start=(ht == 0),
                            stop=(ht == NHT - 1),
                        )
                    nc.vector.tensor_scalar_mul(
                        y[:, dc * 512:(dc + 1) * 512], p2[:],
                        gsb[:, gt:gt + 1])
                    nc.scalar.dma_start(
                        out=outd[t0 + tt * 128:t0 + (tt + 1) * 128,
                                 dc * 512:(dc + 1) * 512],
                        in_=y[:, dc * 512:(dc + 1) * 512])
    nc.compile()
    return nc


# ======================= dense fallback (token-parallel) ====================
TLOC = 1024      # tokens per core
CHUNK = 512      # tokens per hT block
NCH = TLOC // CHUNK
TT = CHUNK // 128        # token tiles per chunk (4)
W1G = H // 512           # 8 w1 DMA groups per expert, each [128, DS, 512]
HQ = 4                   # w2 h-quarters, each 8 h-tiles


def build_nc():
    nc = bacc.Bacc("TRN2", target_bir_lowering=False, debug=False,
                   num_devices=NCORES)
    xh = nc.dram_tensor("xh", [128, DS, TLOC], FR, kind="ExternalInput")
    w1h = nc.dram_tensor("w1h", [E, W1G, 128, DS, 512], FR, kind="ExternalInput")
    w2h = nc.dram_tensor("w2h", [E, DC, HQ, 128, 8, 512], FR, kind="ExternalInput")
    b1h = nc.dram_tensor("b1h", [128, E, NHT], F32, kind="ExternalInput")
    b2h = nc.dram_tensor("b2h", [E, D], FR, kind="ExternalInput")
    # host-computed gates: gh[p, tt_global, e] (token t = tt_global*128 + p)
    gh = nc.dram_tensor("gh", [128, TLOC // 128, E], F32, kind="ExternalInput")
    # transposed gates for the fc2-bias rank-1 term: ght[e, tok]
    ght = nc.dram_tensor("ght", [E, TLOC], FR, kind="ExternalInput")
    outd = nc.dram_tensor("outd", [NCH, 128, TT, DC, 512], F32,
                          kind="ExternalOutput")

    with tile.TileContext(nc) as tc, ExitStack() as ctx:
        const = ctx.enter_context(tc.tile_pool(name="const", bufs=1))
        hpool = ctx.enter_context(tc.tile_pool(name="hT", bufs=1))
        apool = ctx.enter_context(tc.tile_pool(name="oacc", bufs=2))
        w1p = ctx.enter_context(tc.tile_pool(name="w1", bufs=2))
        w2p = ctx.enter_context(tc.tile_pool(name="w2", bufs=2))
        ps1 = ctx.enter_context(tc.tile_pool(name="ps1", bufs=2, space="PSUM"))
        ps2 = ctx.enter_context(tc.tile_pool(name="ps2", bufs=6, space="PSUM"))

        # --- resident tensors ---
        xsb = const.tile([128, DS, TLOC], FR)
        nc.sync.dma_start(out=xsb[:], in_=xh[:, :, :])
        b1sb = const.tile([128, E, NHT], F32)
        nc.sync.dma_start(out=b1sb[:], in_=b1h[:, :, :])
        b2sb = const.tile([E, D], FR)
        nc.sync.dma_start(out=b2sb[:], in_=b2h[:, :])
        gsb = const.tile([128, TLOC // 128, E], F32)
        nc.sync.dma_start(out=gsb[:], in_=gh[:, :, :])
        gtsb = const.tile([E, TLOC], FR)
        nc.sync.dma_start(out=gtsb[:], in_=ght[:, :])

        for c in range(NCH):
            t0 = c * CHUNK
            # init oacc with the fc2 bias term: oacc[t, d] = sum_e g_e(t) b2_e(d)
            oacc = apool.tile([128, TT, DC, 512], F32)
            for tt in range(TT):
                for dc in range(DC):
                    pb = ps2.tile([128, 512], F32, name=f"pb{tt}_{dc}", tag="pst")
                    nc.tensor.matmul(
                        pb[:],
                        lhsT=gtsb[:, t0 + tt * 128: t0 + (tt + 1) * 128],
                        rhs=b2sb[:, dc * 512: (dc + 1) * 512],
                        start=True, stop=True,
                    )
                    nc.vector.tensor_copy(oacc[:, tt, dc, :], pb[:])

            for e in range(E):
                # ---------------- fc1: hT[h, tok] = gelu(w1 @ x + b1) --------
                hT = hpool.tile([128, NHT, CHUNK], FR)
                for wg in range(W1G):  # 8 groups x 4 h-tiles
                    w1t = w1p.tile([128, DS, 512], FR)
                    nc.sync.dma_start(out=w1t[:], in_=w1h[e, wg, :, :, :])
                    for hti in range(4):
                        ht = wg * 4 + hti
                        p1 = ps1.tile([128, 512], F32)
                        for ds in range(DS):
                            nc.tensor.matmul(
                                p1[:, :CHUNK],
                                lhsT=w1t[:, ds, hti * 128: (hti + 1) * 128],
                                rhs=xsb[:, ds, t0: t0 + CHUNK],
                                start=(ds == 0),
                                stop=(ds == DS - 1),
                            )
                        nc.scalar.activation(
                            hT[:, ht, :], p1[:, :CHUNK], AF.Gelu_apprx_tanh,
                            bias=b1sb[:, e, ht: ht + 1],
                        )
                # ---------------- fc2: out[tok, d] += g_e * (hT.T @ w2) ------
                for dc in range(DC):
                    pst = [ps2.tile([128, 512], F32, name=f"pst{_t}", tag="pst")
                           for _t in range(TT)]
                    for hq in range(HQ):
                        w2t = w2p.tile([128, 8, 512], FR)
                        nc.sync.dma_start(out=w2t[:], in_=w2h[e, dc, hq, :, :, :])
                        for hh in range(8):
                            ht = hq * 8 + hh
                            for tt in range(TT):
                                nc.tensor.matmul(
                                    pst[tt][:],
                                    lhsT=hT[:, ht, tt * 128: (tt + 1) * 128],
                                    rhs=w2t[:, hh, :],
                                    start=(hq == 0 and hh == 0),
                                    stop=(hq == HQ - 1 and hh == 7),
                                )
                    for tt in range(TT):
                        nc.vector.scalar_tensor_tensor(
                            out=oacc[:, tt, dc, :],
                            in0=pst[tt][:],
                            scalar=gsb[:, (t0 // 128) + tt, e: e + 1],
                            in1=oacc[:, tt, dc, :],
                            op0=OP.mult,
                            op1=OP.add,
                        )
            nc.sync.dma_start(out=outd[c, :, :, :, :], in_=oacc[:])
    nc.compile()
    return nc


_CACHE = {}


def _get_nc():
    if "nc" not in _CACHE:
        _CACHE["nc"] = build_nc()
    return _CACHE["nc"]


def _get_nc_ep():
    if "ncep" not in _CACHE:
        _CACHE["ncep"] = build_nc_ep()
    return _CACHE["ncep"]


def host_router(x, scale_embeddings, router_w, router_b, scale_idx):
    """Exact-fp32 router matching the reference's op order.

    Returns (gates [T, E] fp32, top2 idx [T, 2], top2 weights [T, 2]).
    """
    f = np.float32
    T = x.shape[0] * x.shape[1]
    xs = (x.astype(f, copy=False)
          + scale_embeddings[int(scale_idx)].astype(f, copy=False)[None, None, :])
    logits = (xs.reshape(T, D) @ router_w.astype(f, copy=False).T
              + router_b.astype(f, copy=False))                    # [T, E]
    # top-2 with jax.lax.top_k tie semantics (lowest index wins)
    neg = -logits
    idx = np.argsort(neg, axis=1, kind="stable")[:, :2]            # [T, 2]
    v = np.take_along_axis(logits, idx, axis=1)
    w = np.exp(v - v[:, :1])
    w = w / w.sum(axis=1, keepdims=True)
    w = w.astype(f)
    gates = np.zeros((T, E), f)
    np.put_along_axis(gates, idx, w, axis=1)
    return gates, idx, w


def make_in_maps_ep(x, scale_embeddings, router_w, router_b,
                    fc1_w, fc1_b, fc2_w, fc2_b, scale_idx):
    """Returns (in_maps, (B, S), flat) or None if any expert overflows CAP.

    flat[2T] are row indices into the concatenated [E*EP_CAP, D] device
    output: flat[:T] = slot-0 row for each token, flat[T:] = slot-1 row.
    """
    x = np.asarray(x, np.float32)
    B, S, _ = x.shape
    T = B * S
    assert x.shape[2] == D
    fc1_w = np.asarray(fc1_w)
    fc1_b = np.asarray(fc1_b)
    fc2_w = np.asarray(fc2_w)
    fc2_b = np.asarray(fc2_b)
    _, top_idx, top_w = host_router(
        x, np.asarray(scale_embeddings), np.asarray(router_w),
        np.asarray(router_b), np.asarray(scale_idx))
    e_all = np.concatenate([top_idx[:, 0], top_idx[:, 1]])         # [2T]
    t_all = np.concatenate([np.arange(T), np.arange(T)])
    w_all = np.concatenate([top_w[:, 0], top_w[:, 1]]).astype(np.float32)
    counts = np.bincount(e_all, minlength=E)
    if counts.max() > EP_CAP:
        return None
    order = np.argsort(e_all, kind="stable")
    off = np.zeros(E + 1, np.int64)
    off[1:] = np.cumsum(counts)
    tok_sorted = t_all[order]
    w_sorted = w_all[order]
    pos = np.empty(2 * T, np.int64)
    pos[order] = np.arange(2 * T)
    flat = e_all.astype(np.int64) * EP_CAP + (pos - off[e_all])

    # host-side fc2-bias term: bterm[t] = sum_s g_s(t) * b2[e_s(t)]
    b2f = fc2_b.astype(np.float32)
    bterm = (top_w[:, 0:1].astype(np.float32) * b2f[top_idx[:, 0]]
             + top_w[:, 1:2].astype(np.float32) * b2f[top_idx[:, 1]])

    xf = x.reshape(T, D)
    in_maps = []
    for e in range(E):
        idx_e = tok_sorted[off[e]:off[e + 1]]
        n_e = len(idx_e)
        xe = np.zeros((EP_CAP, D), np.float32)
        xe[:n_e] = xf[idx_e]
        # xh packed per chunk: block[p, ds, j] = xe[t0+j, ds*128 + p]
        blocks = []
        for (t0, cw) in EP_CHUNKS:
            blk = xe[t0:t0 + cw].reshape(cw, DS, 128).transpose(2, 1, 0)
            blocks.append(blk.reshape(128, DS * cw))
        xhh = np.concatenate(blocks, axis=1).astype(BF16)
        g = np.zeros(EP_CAP, np.float32)
        g[:n_e] = w_sorted[off[e]:off[e + 1]]
        gdd = np.ascontiguousarray(g.reshape(EP_NT, 128).T)        # [128, NT]
        # w1d[p, g, ds, j] = fc1_w[e, g*(H//W1SL)+j, ds*128+p]
        w1dd = (fc1_w[e].reshape(W1SL, H // W1SL, DS, 128)
                .transpose(3, 0, 2, 1).astype(BF16))
        # w2d[p, ht, d] = fc2_w[e, d, ht*128+p]
        w2dd = fc2_w[e].reshape(D, NHT, 128).transpose(2, 1, 0).astype(BF16)
        b1dd = np.ascontiguousarray(
            fc1_b[e].astype(np.float32).reshape(NHT, 128).T)       # [128, NHT]
        in_maps.append({
            "xh": xhh, "w1d": w1dd, "w2d": w2dd, "b1d": b1dd, "gd": gdd,
        })
    return in_maps, (B, S), flat, bterm


def _prep_shared(fc1_w, fc1_b, fc2_w, fc2_b):
    f = np.float32
    w1t = np.ascontiguousarray(fc1_w.transpose(0, 2, 1)).astype(f, copy=False)
    w1h = np.ascontiguousarray(
        w1t.reshape(E, DS, 128, W1G, 512).transpose(0, 3, 2, 1, 4))
    w2t = np.ascontiguousarray(fc2_w.transpose(0, 2, 1)).astype(f, copy=False)
    w2h = np.ascontiguousarray(
        w2t.reshape(E, HQ, 8, 128, DC, 512).transpose(0, 4, 1, 3, 2, 5))
    b1h = np.ascontiguousarray(
        fc1_b.astype(f, copy=False).reshape(E, NHT, 128).transpose(2, 0, 1))
    b2h = np.ascontiguousarray(fc2_b.astype(f, copy=False))
    return w1h, w2h, b1h, b2h


def make_in_maps(x, scale_embeddings, router_w, router_b,
                 fc1_w, fc1_b, fc2_w, fc2_b, scale_idx):
    x = np.asarray(x, np.float32)
    B, S, _ = x.shape
    T = B * S
    assert T == NCORES * TLOC and x.shape[2] == D
    w1h, w2h, b1h, b2h = _prep_shared(
        np.asarray(fc1_w), np.asarray(fc1_b),
        np.asarray(fc2_w), np.asarray(fc2_b))
    gates, _, _ = host_router(x, np.asarray(scale_embeddings),
                              np.asarray(router_w), np.asarray(router_b),
                              np.asarray(scale_idx))
    xf = x.reshape(T, D)
    in_maps = []
    for i in range(NCORES):
        xloc = xf[i * TLOC:(i + 1) * TLOC]                       # [TLOC, D]
        xT = np.ascontiguousarray(xloc.T)                        # [D, TLOC]
        xhh = np.ascontiguousarray(
            xT.reshape(DS, 128, TLOC).transpose(1, 0, 2))        # [128, DS, TLOC]
        gloc = gates[i * TLOC:(i + 1) * TLOC]                    # [TLOC, E]
        ghh = np.ascontiguousarray(
            gloc.reshape(TLOC // 128, 128, E).transpose(1, 0, 2))
        ght = np.ascontiguousarray(gloc.T)                       # [E, TLOC]
        in_maps.append({
            "xh": xhh, "w1h": w1h, "w2h": w2h, "b1h": b1h,
            "b2h": b2h, "gh": ghh, "ght": ght,
        })
    return in_maps, (B, S)


def kernel(x, scale_embeddings, router_w, router_b,
           fc1_w, fc1_b, fc2_w, fc2_b, scale_idx):
    args = (x, scale_embeddings, router_w, router_b,
            fc1_w, fc1_b, fc2_w, fc2_b, scale_idx)
    ep = make_in_maps_ep(*args)
    if ep is not None:
        in_maps, (B, S), flat, bterm = ep
        T = B * S
        nc = _get_nc_ep()
        res = run_bass_kernel_spmd(nc, in_maps, core_ids=list(range(NCORES)))
        Y = np.stack([res.results[i]["outd"] for i in range(NCORES)])
        Yf = Y.reshape(E * EP_CAP, D)
        out = Yf[flat[:T]] + Yf[flat[T:]] + bterm
        return out.reshape(B, S, D)
    # capacity overflow (practically impossible): dense fallback
    in_maps, (B, S) = make_in_maps(*args)
    nc = _get_nc()
    res = run_bass_kernel_spmd(nc, in_maps, core_ids=list(range(NCORES)))
    parts = []
    for i in range(NCORES):
        o = res.results[i]["outd"]                               # [NCH,128,TT,DC,512]
        parts.append(o.transpose(0, 2, 1, 3, 4).reshape(TLOC, D))
    return np.concatenate(parts, 0).reshape(B, S, D)


# revision 34
# speedup vs baseline: 1.0292x; 1.0037x over previous
"""MoE FFN (8 experts, top-2) Trainium2 Bass kernel.

Strategy: EXPERT-parallel across the 8 cores. Core e owns expert e's
weights, resident in SBUF as bf16 (w1 64KiB/partition + w2 64KiB/partition),
and processes every token routed to expert e (CAP=2176 padded slots; actual
max expert load for this input distribution is ~2120 of mean 2048). The tiny
router (0.06% of FLOPs) runs on host in exact fp32 (matching the reference's
op order so top-2 selection is stable); the host also gathers/transposes each
expert's tokens and scatter-gathers the outputs back, so the device kernel is
a pure dense FFN: hT = gelu_tanh(w1 @ x + b1) -> y = gate * (hT.T @ w2 + b2).

This beats token-parallel (capacity 384 per (core,expert) = 3072 slots/core,
1.5x padding) at 2176 slots/core (1.06x padding), removes all device-side
gathers/transposes/scatters, and streams weights exactly once (resident).

Everything stays in [feature, token] layout on chip: fc1 consumes w1T tiles
as stationary and xT as moving; fc2 consumes hT tiles as stationary and w2T
as moving, producing [token, d] PSUM tiles so the per-token gate is a
per-partition scalar multiply (one DVE op). b2 is added via a K=1 ones
matmul folded into the fc2 PSUM accumulation. Matmul operands are bf16
(fp32 PSUM accumulate): rel err ~5e-3, well under the 2e-2 gate.
"""

import numpy as np
from contextlib import ExitStack

import ml_dtypes

import concourse.bass as bass
import concourse.bacc as bacc
import concourse.tile as tile
from concourse import mybir
from concourse.bass_utils import run_bass_kernel_spmd

BF16 = ml_dtypes.bfloat16
FR = mybir.dt.float32r
BF = mybir.dt.bfloat16
F32 = mybir.dt.float32
AF = mybir.ActivationFunctionType
OP = mybir.AluOpType

NCORES = 8
E = 8            # experts
D = 1024         # model dim
H = 4096         # hidden dim
DS = D // 128    # d sub-blocks (8)
NHT = H // 128   # h tiles (32)
DC = D // 512    # output d chunks (2)

# ---- expert-parallel kernel geometry ----
EP_CAP = 2176                  # token capacity per expert (17 tiles of 128)
EP_NT = EP_CAP // 128          # 17
# All chunk widths >= 256 so fc1 matmul dur >= LDWEIGHTS dur (stays hidden).
# 512 first: fc1 chunk 0 must run ~55us so the w2 transfer (queued behind
# w1 on the sync queue) lands before fc2 chunk 0 needs it.
EP_CHUNKS = [(0, 512), (512, 512), (1024, 512), (1536, 384), (1920, 256)]
W1SL = 16                      # w1 load slices (2 h-tiles each)


def build_nc_ep():
    """Expert-parallel dense FFN: one expert per core, weights resident."""
    nc = bacc.Bacc("TRN2", target_bir_lowering=False, debug=False,
                   num_devices=NCORES)
    # xh is packed per chunk ([ds, tok] blocks) and w1d per 512-h group so
    # every DMA slice is contiguous per partition (8KB descriptors; a
    # last-dim slice of [128, DS, H] would issue 1KB descriptors and run
    # ~8x slower, gating the kernel start).
    xh = nc.dram_tensor("xh", [128, DS * EP_CAP], BF, kind="ExternalInput")
    w1d = nc.dram_tensor("w1d", [128, W1SL, DS, H // W1SL], BF,
                         kind="ExternalInput")
    w2d = nc.dram_tensor("w2d", [128, NHT, D], BF, kind="ExternalInput")
    b1d = nc.dram_tensor("b1d", [128, NHT], F32, kind="ExternalInput")
    gd = nc.dram_tensor("gd", [128, EP_NT], F32, kind="ExternalInput")
    outd = nc.dram_tensor("outd", [EP_CAP, D], F32, kind="ExternalOutput")

    with tile.TileContext(nc) as tc, ExitStack() as ctx:
        const = ctx.enter_context(tc.tile_pool(name="const", bufs=1))
        xp = ctx.enter_context(tc.tile_pool(name="xp", bufs=2))
        hp = ctx.enter_context(tc.tile_pool(name="hp", bufs=1))
        yp = ctx.enter_context(tc.tile_pool(name="yp", bufs=4))
        ps1 = ctx.enter_context(tc.tile_pool(name="ps1", bufs=3, space="PSUM"))
        ps2 = ctx.enter_context(tc.tile_pool(name="ps2", bufs=4, space="PSUM"))
        wps = ctx.enter_context(tc.tile_pool(name="wps", bufs=1, space="PSUM"))

        # Startup DMA plan — everything bulk goes on the SYNC queue, in
        # consumption order. Measured queue behavior: a single queue alone
        # sustains ~450GB/s, but two concurrent queues split by DESCRIPTOR
        # count, so a big-descriptor stream on one queue starves small
        # descriptors on the other (observed 400 vs 50 GB/s). Also, a
        # dma_start HOLDS its engine's FIFO while waiting for a ring slot,
        # so bulk loads must not sit on the Activation FIFO ahead of the
        # gelus. Hence:
        #   sync: x0 | w1 as 8 slice dmas (land every ~2.4us, fc1 consumes
        #         one per ~6.9us) | w2 | gates | x prefetches (chunk loop)
        #   act:  b1 (tiny) | y writebacks
        # Dependencies are full-completion per dma instruction, and a tile
        # written by k dmas blocks readers until all k land — hence one
        # tile per w1 slice.
        # The two transfers gating the first real matmul (x chunk 0 and w1
        # slice 0) run on SEPARATE queues in parallel; everything else
        # follows the single-queue plan above.
        xt0 = xp.tile([128, DS, 512], BF, tag="xt")
        nc.sync.dma_start(out=xt0[:], in_=xh[:, 0:DS * 512])
        w1s = []
        for i in range(W1SL):
            t = const.tile([128, DS, H // W1SL], BF, tag=f"w1_{i}")
            if i == 0:
                nc.scalar.dma_start(out=t[:], in_=w1d[:, i, :, :])
            else:
                nc.sync.dma_start(out=t[:], in_=w1d[:, i, :, :])
            w1s.append(t)
        b1sb = const.tile([128, NHT], F32)
        nc.scalar.dma_start(out=b1sb[:], in_=b1d[:, :])
        w2sb = const.tile([128, NHT, D], BF)
        nc.sync.dma_start(out=w2sb[:], in_=w2d[:, :, :])
        gsb = const.tile([128, EP_NT], F32)
        nc.sync.dma_start(out=gsb[:], in_=gd[:, :])

        # Warm the PE's HAM clock gate during the startup DMA window: memset
        # a scratch tile (DVE, no DMA dependency) and run dummy N=128
        # matmuls on it. They issue right after the preamble and bridge the
        # ~7.5us until x0 + the first w1 slice land (~36 at the cold clock,
        # then ~56ns each warm), so the real stream starts at 2.4 GHz with
        # no HAM re-throttle.
        wsrc = const.tile([128, 128], BF)
        nc.vector.memset(wsrc[:], 0.0)
        warm = wps.tile([128, 128], F32, tag="warm")
        for _ in range(64):
            nc.tensor.matmul(warm[:], lhsT=wsrc[:, :], rhs=wsrc[:, :],
                             start=True, stop=True)

        for ci, (t0, cw) in enumerate(EP_CHUNKS):
            if ci == 0:
                xt = xt0
            else:
                xt = xp.tile([128, DS, cw], BF, tag="xt")
                nc.sync.dma_start(out=xt[:],
                                  in_=xh[:, DS * t0:DS * (t0 + cw)])
            # ---------------- fc1: hT[h, tok] = gelu(w1 @ x + b1) --------
            hT = hp.tile([128, NHT, cw], BF, tag="hT")
            for ht in range(NHT):
                p1 = ps1.tile([128, cw], F32, tag="p1")
                for ds in range(DS):
                    nc.tensor.matmul(
                        p1[:],
                        lhsT=w1s[ht // 2][:, ds,
                                          (ht % 2) * 128:(ht % 2 + 1) * 128],
                        rhs=xt[:, ds, :],
                        start=(ds == 0),
                        stop=(ds == DS - 1),
                    )
                nc.scalar.activation(
                    hT[:, ht, :], p1[:], AF.Gelu_apprx_tanh,
                    bias=b1sb[:, ht:ht + 1],
                )
            # ------------ fc2: y[tok, d] = g * (hT.T @ w2)  (b2 on host) --
            # y is written back per d-half so the dc=0 transfer overlaps the
            # dc=1 matmul group (shortens the post-stream drain).
            for tt in range(cw // 128):
                gt = t0 // 128 + tt
                y = yp.tile([128, D], F32, tag="y")
                for dc in range(DC):
                    p2 = ps2.tile([128, 512], F32, tag="p2")
                    for ht in range(NHT):
                        nc.tensor.matmul(
                            p2[:],
                            lhsT=hT[:, ht, tt * 128:(tt + 1) * 128],
                            rhs=w2sb[:, ht, dc * 512:(dc + 1) * 512],
                            start=(ht == 0),
                            stop=(ht == NHT - 1),
                        )
                    nc.vector.tensor_scalar_mul(
                        y[:, dc * 512:(dc + 1) * 512], p2[:],
                        gsb[:, gt:gt + 1])
                    nc.scalar.dma_start(
                        out=outd[t0 + tt * 128:t0 + (tt + 1) * 128,
                                 dc * 512:(dc + 1) * 512],
                        in_=y[:, dc * 512:(dc + 1) * 512])
    nc.compile()
    return nc


# ======================= dense fallback (token-parallel) ====================
TLOC = 1024      # tokens per core
CHUNK = 512      # tokens per hT block
NCH = TLOC // CHUNK
TT = CHUNK // 128        # token tiles per chunk (4)
W1G = H // 512           # 8 w1 DMA groups per expert, each [128, DS, 512]
HQ = 4                   # w2 h-quarters, each 8 h-tiles


def build_nc():
    nc = bacc.Bacc("TRN2", target_bir_lowering=False, debug=False,
                   num_devices=NCORES)
    xh = nc.dram_tensor("xh", [128, DS, TLOC], FR, kind="ExternalInput")
    w1h = nc.dram_tensor("w1h", [E, W1G, 128, DS, 512], FR, kind="ExternalInput")
    w2h = nc.dram_tensor("w2h", [E, DC, HQ, 128, 8, 512], FR, kind="ExternalInput")
    b1h = nc.dram_tensor("b1h", [128, E, NHT], F32, kind="ExternalInput")
    b2h = nc.dram_tensor("b2h", [E, D], FR, kind="ExternalInput")
    # host-computed gates: gh[p, tt_global, e] (token t = tt_global*128 + p)
    gh = nc.dram_tensor("gh", [128, TLOC // 128, E], F32, kind="ExternalInput")
    # transposed gates for the fc2-bias rank-1 term: ght[e, tok]
    ght = nc.dram_tensor("ght", [E, TLOC], FR, kind="ExternalInput")
    outd = nc.dram_tensor("outd", [NCH, 128, TT, DC, 512], F32,
                          kind="ExternalOutput")

    with tile.TileContext(nc) as tc, ExitStack() as ctx:
        const = ctx.enter_context(tc.tile_pool(name="const", bufs=1))
        hpool = ctx.enter_context(tc.tile_pool(name="hT", bufs=1))
        apool = ctx.enter_context(tc.tile_pool(name="oacc", bufs=2))
        w1p = ctx.enter_context(tc.tile_pool(name="w1", bufs=2))
        w2p = ctx.enter_context(tc.tile_pool(name="w2", bufs=2))
        ps1 = ctx.enter_context(tc.tile_pool(name="ps1", bufs=2, space="PSUM"))
        ps2 = ctx.enter_context(tc.tile_pool(name="ps2", bufs=6, space="PSUM"))

        # --- resident tensors ---
        xsb = const.tile([128, DS, TLOC], FR)
        nc.sync.dma_start(out=xsb[:], in_=xh[:, :, :])
        b1sb = const.tile([128, E, NHT], F32)
        nc.sync.dma_start(out=b1sb[:], in_=b1h[:, :, :])
        b2sb = const.tile([E, D], FR)
        nc.sync.dma_start(out=b2sb[:], in_=b2h[:, :])
        gsb = const.tile([128, TLOC // 128, E], F32)
        nc.sync.dma_start(out=gsb[:], in_=gh[:, :, :])
        gtsb = const.tile([E, TLOC], FR)
        nc.sync.dma_start(out=gtsb[:], in_=ght[:, :])

        for c in range(NCH):
            t0 = c * CHUNK
            # init oacc with the fc2 bias term: oacc[t, d] = sum_e g_e(t) b2_e(d)
            oacc = apool.tile([128, TT, DC, 512], F32)
            for tt in range(TT):
                for dc in range(DC):
                    pb = ps2.tile([128, 512], F32, name=f"pb{tt}_{dc}", tag="pst")
                    nc.tensor.matmul(
                        pb[:],
                        lhsT=gtsb[:, t0 + tt * 128: t0 + (tt + 1) * 128],
                        rhs=b2sb[:, dc * 512: (dc + 1) * 512],
                        start=True, stop=True,
                    )
                    nc.vector.tensor_copy(oacc[:, tt, dc, :], pb[:])

            for e in range(E):
                # ---------------- fc1: hT[h, tok] = gelu(w1 @ x + b1) --------
                hT = hpool.tile([128, NHT, CHUNK], FR)
                for wg in range(W1G):  # 8 groups x 4 h-tiles
                    w1t = w1p.tile([128, DS, 512], FR)
                    nc.sync.dma_start(out=w1t[:], in_=w1h[e, wg, :, :, :])
                    for hti in range(4):
                        ht = wg * 4 + hti
                        p1 = ps1.tile([128, 512], F32)
                        for ds in range(DS):
                            nc.tensor.matmul(
                                p1[:, :CHUNK],
                                lhsT=w1t[:, ds, hti * 128: (hti + 1) * 128],
                                rhs=xsb[:, ds, t0: t0 + CHUNK],
                                start=(ds == 0),
                                stop=(ds == DS - 1),
                            )
                        nc.scalar.activation(
                            hT[:, ht, :], p1[:, :CHUNK], AF.Gelu_apprx_tanh,
                            bias=b1sb[:, e, ht: ht + 1],
                        )
                # ---------------- fc2: out[tok, d] += g_e * (hT.T @ w2) ------
                for dc in range(DC):
                    pst = [ps2.tile([128, 512], F32, name=f"pst{_t}", tag="pst")
                           for _t in range(TT)]
                    for hq in range(HQ):
                        w2t = w2p.tile([128, 8, 512], FR)
                        nc.sync.dma_start(out=w2t[:], in_=w2h[e, dc, hq, :, :, :])
                        for hh in range(8):
                            ht = hq * 8 + hh
                            for tt in range(TT):
                                nc.tensor.matmul(
                                    pst[tt][:],
                                    lhsT=hT[:, ht, tt * 128: (tt + 1) * 128],
                                    rhs=w2t[:, hh, :],
                                    start=(hq == 0 and hh == 0),
                                    stop=(hq == HQ - 1 and hh == 7),
                                )
                    for tt in range(TT):
                        nc.vector.scalar_tensor_tensor(
                            out=oacc[:, tt, dc, :],
                            in0=pst[tt][:],
                            scalar=gsb[:, (t0 // 128) + tt, e: e + 1],
                            in1=oacc[:, tt, dc, :],
                            op0=OP.mult,
                            op1=OP.add,
                        )
            nc.sync.dma_start(out=outd[c, :, :, :, :], in_=oacc[:])
    nc.compile()
    return nc


_CACHE = {}


def _get_nc():
    if "nc" not in _CACHE:
        _CACHE["nc"] = build_nc()
    return _CACHE["nc"]


def _get_nc_ep():
    if "ncep" not in _CACHE:
        _CACHE["ncep"] = build_nc_ep()
    return _CACHE["ncep"]


def host_router(x, scale_embeddings, router_w, router_b, scale_idx):
    """Exact-fp32 router matching the reference's op order.

    Returns (gates [T, E] fp32, top2 idx [T, 2], top2 weights [T, 2]).
    """
    f = np.float32
    T = x.shape[0] * x.shape[1]
    xs = (x.astype(f, copy=False)
          + scale_embeddings[int(scale_idx)].astype(f, copy=False)[None, None, :])
    logits = (xs.reshape(T, D) @ router_w.astype(f, copy=False).T
              + router_b.astype(f, copy=False))                    # [T, E]
    # top-2 with jax.lax.top_k tie semantics (lowest index wins)
    neg = -logits
    idx = np.argsort(neg, axis=1, kind="stable")[:, :2]            # [T, 2]
    v = np.take_along_axis(logits, idx, axis=1)
    w = np.exp(v - v[:, :1])
    w = w / w.sum(axis=1, keepdims=True)
    w = w.astype(f)
    gates = np.zeros((T, E), f)
    np.put_along_axis(gates, idx, w, axis=1)
    return gates, idx, w


def make_in_maps_ep(x, scale_embeddings, router_w, router_b,
                    fc1_w, fc1_b, fc2_w, fc2_b, scale_idx):
    """Returns (in_maps, (B, S), flat) or None if any expert overflows CAP.

    flat[2T] are row indices into the concatenated [E*EP_CAP, D] device
    output: flat[:T] = slot-0 row for each token, flat[T:] = slot-1 row.
    """
    x = np.asarray(x, np.float32)
    B, S, _ = x.shape
    T = B * S
    assert x.shape[2] == D
    fc1_w = np.asarray(fc1_w)
    fc1_b = np.asarray(fc1_b)
    fc2_w = np.asarray(fc2_w)
    fc2_b = np.asarray(fc2_b)
    _, top_idx, top_w = host_router(
        x, np.asarray(scale_embeddings), np.asarray(router_w),
        np.asarray(router_b), np.asarray(scale_idx))
    e_all = np.concatenate([top_idx[:, 0], top_idx[:, 1]])         # [2T]
    t_all = np.concatenate([np.arange(T), np.arange(T)])
    w_all = np.concatenate([top_w[:, 0], top_w[:, 1]]).astype(np.float32)
    counts = np.bincount(e_all, minlength=E)
    if counts.max() > EP_CAP:
        return None
    order = np.argsort(e_all, kind="stable")
    off = np.zeros(E + 1, np.int64)
    off[1:] = np.cumsum(counts)
    tok_sorted = t_all[order]
    w_sorted = w_all[order]
    pos = np.empty(2 * T, np.int64)
    pos[order] = np.arange(2 * T)
    flat = e_all.astype(np.int64) * EP_CAP + (pos - off[e_all])

    # host-side fc2-bias term: bterm[t] = sum_s g_s(t) * b2[e_s(t)]
    b2f = fc2_b.astype(np.float32)
    bterm = (top_w[:, 0:1].astype(np.float32) * b2f[top_idx[:, 0]]
             + top_w[:, 1:2].astype(np.float32) * b2f[top_idx[:, 1]])

    xf = x.reshape(T, D)
    in_maps = []
    for e in range(E):
        idx_e = tok_sorted[off[e]:off[e + 1]]
        n_e = len(idx_e)
        xe = np.zeros((EP_CAP, D), np.float32)
        xe[:n_e] = xf[idx_e]
        # xh packed per chunk: block[p, ds, j] = xe[t0+j, ds*128 + p]
        blocks = []
        for (t0, cw) in EP_CHUNKS:
            blk = xe[t0:t0 + cw].reshape(cw, DS, 128).transpose(2, 1, 0)
            blocks.append(blk.reshape(128, DS * cw))
        xhh = np.concatenate(blocks, axis=1).astype(BF16)
        g = np.zeros(EP_CAP, np.float32)
        g[:n_e] = w_sorted[off[e]:off[e + 1]]
        gdd = np.ascontiguousarray(g.reshape(EP_NT, 128).T)        # [128, NT]
        # w1d[p, g, ds, j] = fc1_w[e, g*(H//W1SL)+j, ds*128+p]
        w1dd = (fc1_w[e].reshape(W1SL, H // W1SL, DS, 128)
                .transpose(3, 0, 2, 1).astype(BF16))
        # w2d[p, ht, d] = fc2_w[e, d, ht*128+p]
        w2dd = fc2_w[e].reshape(D, NHT, 128).transpose(2, 1, 0).astype(BF16)
        b1dd = np.ascontiguousarray(
            fc1_b[e].astype(np.float32).reshape(NHT, 128).T)       # [128, NHT]
        in_maps.append({
            "xh": xhh, "w1d": w1dd, "w2d": w2dd, "b1d": b1dd, "gd": gdd,
        })
    return in_maps, (B, S), flat, bterm


def _prep_shared(fc1_w, fc1_b, fc2_w, fc2_b):
    f = np.float32
    w1t = np.ascontiguousarray(fc1_w.transpose(0, 2, 1)).astype(f, copy=False)
    w1h = np.ascontiguousarray(
        w1t.reshape(E, DS, 128, W1G, 512).transpose(0, 3, 2, 1, 4))
    w2t = np.ascontiguousarray(fc2_w.transpose(0, 2, 1)).astype(f, copy=False)
    w2h = np.ascontiguousarray(
        w2t.reshape(E, HQ, 8, 128, DC, 512).transpose(0, 4, 1, 3, 2, 5))
    b1h = np.ascontiguousarray(
        fc1_b.astype(f, copy=False).reshape(E, NHT, 128).transpose(2, 0, 1))
    b2h = np.ascontiguousarray(fc2_b.astype(f, copy=False))
    return w1h, w2h, b1h, b2h


def make_in_maps(x, scale_embeddings, router_w, router_b,
                 fc1_w, fc1_b, fc2_w, fc2_b, scale_idx):
    x = np.asarray(x, np.float32)
    B, S, _ = x.shape
    T = B * S
    assert T == NCORES * TLOC and x.shape[2] == D
    w1h, w2h, b1h, b2h = _prep_shared(
        np.asarray(fc1_w), np.asarray(fc1_b),
        np.asarray(fc2_w), np.asarray(fc2_b))
    gates, _, _ = host_router(x, np.asarray(scale_embeddings),
                              np.asarray(router_w), np.asarray(router_b),
                              np.asarray(scale_idx))
    xf = x.reshape(T, D)
    in_maps = []
    for i in range(NCORES):
        xloc = xf[i * TLOC:(i + 1) * TLOC]                       # [TLOC, D]
        xT = np.ascontiguousarray(xloc.T)                        # [D, TLOC]
        xhh = np.ascontiguousarray(
            xT.reshape(DS, 128, TLOC).transpose(1, 0, 2))        # [128, DS, TLOC]
        gloc = gates[i * TLOC:(i + 1) * TLOC]                    # [TLOC, E]
        ghh = np.ascontiguousarray(
            gloc.reshape(TLOC // 128, 128, E).transpose(1, 0, 2))
        ght = np.ascontiguousarray(gloc.T)                       # [E, TLOC]
        in_maps.append({
            "xh": xhh, "w1h": w1h, "w2h": w2h, "b1h": b1h,
            "b2h": b2h, "gh": ghh, "ght": ght,
        })
    return in_maps, (B, S)


def kernel(x, scale_embeddings, router_w, router_b,
           fc1_w, fc1_b, fc2_w, fc2_b, scale_idx):
    args = (x, scale_embeddings, router_w, router_b,
            fc1_w, fc1_b, fc2_w, fc2_b, scale_idx)
    ep = make_in_maps_ep(*args)
    if ep is not None:
        in_maps, (B, S), flat, bterm = ep
        T = B * S
        nc = _get_nc_ep()
        res = run_bass_kernel_spmd(nc, in_maps, core_ids=list(range(NCORES)))
        Y = np.stack([res.results[i]["outd"] for i in range(NCORES)])
        Yf = Y.reshape(E * EP_CAP, D)
        out = Yf[flat[:T]] + Yf[flat[T:]] + bterm
        return out.reshape(B, S, D)
    # capacity overflow (practically impossible): dense fallback
    in_maps, (B, S) = make_in_maps(*args)
    nc = _get_nc()
    res = run_bass_kernel_spmd(nc, in_maps, core_ids=list(range(NCORES)))
    parts = []
    for i in range(NCORES):
        o = res.results[i]["outd"]                               # [NCH,128,TT,DC,512]
        parts.append(o.transpose(0, 2, 1, 3, 4).reshape(TLOC, D))
    return np.concatenate(parts, 0).reshape(B, S, D)
